# revision 60
# baseline (speedup 1.0000x reference)
"""GCN layer (PyG GCNConv semantics) on 8 Trainium2 NeuronCores via Bass.

v3 — banded gather+reduce device algorithm (proven on HW) with a
wall-clock-oriented host/transfer pipeline:

  host:   deg/dinv via one weighted bincount; edges ranked within
          (dst node, src section) groups via a single int32 radix
          argsort; slot/idx arrays built with two global scatters.
          x shipped as fp16 rows; idx shipped un-replicated ([16, L])
          and replicated to the 8 gpsimd cores on-device.
  device: h' = (x @ W^T) * dinv[src] via PE transpose + matmul (fp16),
          AllGather h' -> full node table (natural row order, 256B
          pitch), per-piece dma_gather of h'[src] into equal-K bands,
          * ew, segmented vector reduce per 128-dst tile,
          out = relu((acc + h'_own) * dinv + b)   (self loops via the
          h'_own add - no self slots), fp16 output in natural order.
"""

import os
import sys

for _p in ("/opt/trn_rl_repo",):
    if _p not in sys.path and os.path.isdir(_p):
        sys.path.insert(0, _p)

import numpy as np

import concourse.bass as bass
import concourse.mybir as mybir
import concourse.tile as tile
from concourse import bacc
from concourse import masks

# ---------------------------------------------------------------- config

P = 128           # partitions
D = 64            # feature dim (in == out)
CORES = 8
NSEC = 4          # int16-addressable table sections
MAX_PIECE_COLS = 192


class Cfg:
    def __init__(self, n_nodes, n_cores=CORES):
        assert n_nodes % n_cores == 0
        self.n = n_nodes
        self.cores = n_cores
        self.npc = n_nodes // n_cores                 # real nodes per core
        self.tiles = (self.npc + P - 1) // P          # 128-node tiles per core
        self.npcp = self.tiles * P                    # padded nodes per core
        self.nrows = self.npcp * n_cores              # table rows
        assert self.nrows % NSEC == 0
        self.srows = self.nrows // NSEC               # rows per section
        assert self.srows <= 32768, "section exceeds int16 index range"


def _pieces_from_kt(kt):
    """Greedy grouping of tiles into pieces with <= MAX_PIECE_COLS slot
    columns (NSEC * sum of widths). Deterministic; shared host/device."""
    T = len(kt)
    pieces = []
    t0 = 0
    while t0 < T:
        t1, ws = t0, 0
        while t1 < T and NSEC * (ws + kt[t1]) <= MAX_PIECE_COLS:
            ws += kt[t1]
            t1 += 1
        assert t1 > t0, f"tile {t0} K={kt[t0]} exceeds piece budget"
        pieces.append((t0, t1, int(ws)))
        t0 = t1
    return pieces


# ---------------------------------------------------------------- host prep

def host_prep(cfg, x, edge_index, edge_weight, W, b, stage1_cb=None,
              half_cb=None):
    """Light preprocessing. Returns (arrays dict, kt tuple). Arrays are
    concatenated across cores along axis 0 (the shard_map axis).
    stage1_cb, if given, receives the edge-independent arrays early so
    their upload can overlap the edge analysis."""
    n, npc, npcp, T = cfg.n, cfg.npc, cfg.npcp, cfg.tiles
    C, SR = cfg.cores, cfg.srows

    src = np.asarray(edge_index[0]).astype(np.int32, copy=False)
    dst = np.asarray(edge_index[1]).astype(np.int32, copy=False)
    ew = np.asarray(edge_weight, dtype=np.float32)
    x = np.asarray(x, dtype=np.float32)
    W = np.asarray(W, dtype=np.float32)
    b = np.asarray(b, dtype=np.float32)
    E = src.shape[0]

    x16 = _buf("x16", (n, D), np.float16)   # [n, D] = per-core [npc, D]
    np.copyto(x16, x, casting="unsafe")

    # deg / dinv on host
    deg = np.bincount(dst, weights=ew, minlength=n).astype(np.float32) + 1.0
    dinv = 1.0 / np.sqrt(deg)
    dv = np.zeros((C, npcp), np.float32)
    dv[:, :npc] = dinv.reshape(C, npc)
    dv = np.ascontiguousarray(dv.reshape(C, T, P).transpose(0, 2, 1))

    wt16 = np.tile(np.ascontiguousarray(W.T).astype(np.float16), (C, 1))
    b64 = np.tile(b[None, :].astype(np.float32), (C * P, 1))

    stage1 = dict(
        xr=x16,
        wt=wt16,
        b64=b64,
        dinv=dv.reshape(C * P, T),
    )
    if stage1_cb is not None:
        stage1_cb(stage1)

    # ---- per (dst node, section) ranks (all scratch in reused buffers)
    i32 = np.int32

    def ib(nm):
        return _buf(nm, (E,), i32)

    q, l = ib("q"), ib("l")
    np.floor_divide(src, npc, out=q)
    np.multiply(q, npc, out=l)
    np.subtract(src, l, out=l)
    r_src = ib("r_src")
    np.multiply(q, npcp, out=r_src)
    np.add(r_src, l, out=r_src)                # natural global table row
    g = ib("g")
    np.floor_divide(r_src, SR, out=g)
    tmp = ib("tmp")
    np.multiply(g, SR, out=tmp)
    np.subtract(r_src, tmp, out=tmp)
    rloc = _buf("rloc", (E,), np.int16)
    np.copyto(rloc, tmp, casting="unsafe")
    cd, ld = ib("cd"), ib("ld")
    np.floor_divide(dst, npc, out=cd)
    np.multiply(cd, npc, out=ld)
    np.subtract(dst, ld, out=ld)
    t_e, p_e = ib("t_e"), ib("p_e")
    np.right_shift(ld, 7, out=t_e)
    np.bitwise_and(ld, P - 1, out=p_e)
    key = ib("key")                            # (cd*npcp + ld)*NSEC + g
    np.multiply(cd, npcp, out=key)
    np.add(key, ld, out=key)
    np.multiply(key, NSEC, out=key)
    np.add(key, g, out=key)

    # stable group-by-key via scipy's O(E) counting sort (much faster
    # than np.argsort on a 21-bit key)
    from scipy.sparse import _sparsetools
    nk = C * npcp * NSEC
    ids = _buf("ids", (E,), i32)
    if ids[E - 1] != E - 1:
        ids[:] = np.arange(E, dtype=i32)
    indptr = _buf("indptr", (nk + 1,), i32)
    bj, order = ib("bj"), ib("order")
    _sparsetools.coo_tocsr(nk, E, E, key, ids, ids, indptr, bj, order)
    cnt = np.diff(indptr)
    kt = cnt.reshape(C, T, P, NSEC).max(axis=(0, 2, 3))
    kt = np.maximum(kt, 1)
    kt = tuple(int(v) for v in kt)

    pieces = _pieces_from_kt(kt)
    piece_of_t = np.zeros(T, np.int64)
    base_in_piece = np.zeros(T, np.int64)
    piece_colbase = np.zeros(len(pieces), np.int64)
    colcur = 0
    for pi, (a, bnd, ws) in enumerate(pieces):
        piece_colbase[pi] = colcur
        off = 0
        for t in range(a, bnd):
            piece_of_t[t] = pi
            base_in_piece[t] = off
            off += kt[t]
        colcur += NSEC * ws
    s_cols = int(colcur)
    ws_of_t = np.array([pieces[piece_of_t[t]][2] for t in range(T)], np.int64)
    colbase_t = (piece_colbase[piece_of_t] + base_in_piece).astype(np.int32)
    ws_t32 = ws_of_t.astype(np.int32)

    starts = indptr[:-1]
    ko, so = ib("ko"), ib("so")
    np.take(key, order, out=ko)
    np.take(starts, ko, out=so)
    np.subtract(ids, so, out=so)
    ranks = ib("ranks")
    ranks[order] = so

    col = ib("col")                  # colbase_t[t_e] + g*ws_t[t_e] + ranks
    np.take(colbase_t, t_e, out=col)
    np.take(ws_t32, t_e, out=tmp)
    np.multiply(tmp, g, out=tmp)
    np.add(col, tmp, out=col)
    np.add(col, ranks, out=col)
    epos = ib("epos")
    np.multiply(col, P, out=epos)
    np.add(epos, p_e, out=epos)

    ew16 = _buf("ew16", (E,), np.float16)
    np.copyto(ew16, ew, casting="unsafe")

    # split at a piece boundary: scatter/pack/upload the first piece
    # range while the second is still being scattered (keeps the
    # host->device tunnel busy). half_cb(blob_bytes) uploads a half.
    psplit = len(pieces) // 2
    if half_cb is not None and psplit >= 1:
        colsplit = int(sum(NSEC * pieces[q_][2] for q_ in range(psplit)))
        thr = pieces[psplit][0] * P          # first tile of second half
        mA = ld < thr
        halves = []
        for h, (c0, c1) in enumerate([(0, colsplit), (colsplit, s_cols)]):
            ih = np.flatnonzero(mA if h == 0 else ~mA)
            w = c1 - c0
            ewh = _buf(f"ew_slots{h}", (C, P, w), np.float16, zero=True)
            idxh = _buf(f"idx_lin{h}", (C, w * P), np.int16, zero=True)
            cdh = np.take(cd, ih)
            ph = np.take(p_e, ih)
            colh = np.take(col, ih)
            if c0:
                colh = colh - c0
            ewh[cdh, ph, colh] = np.take(ew16, ih)
            idxh[cdh, colh * P + ph] = np.take(rloc, ih)
            idxwh = _buf(f"idxw{h}", (C, 16, w * 8), np.int16)
            np.copyto(idxwh, idxh.reshape(C, w * 8, 16).transpose(0, 2, 1))
            halves.append((w, _pack(
                dict(i=idxwh.reshape(C * 16, w * 8),
                     e=ewh.reshape(C * P, w)),
                ("i", "e"), C, tag=f"blobh{h}")))
            half_cb(h, halves[-1][1])
        return (colsplit, s_cols), kt

    if stage1_cb is not None:
        # hw path: scatter/wrap directly into the upload blob (saves a
        # 21MB pack copy). Per-core layout: [idxw bytes | ew bytes].
        A = s_cols * 256                    # 16 * s_cols*8 * 2B
        NB2 = A + s_cols * 256              # + 128 * s_cols * 2B
        blob2 = _buf("blob2d", (C, NB2), np.uint8)
        idxw_v = np.ndarray((C, 16, s_cols * 8), np.int16, buffer=blob2,
                            offset=0, strides=(NB2, s_cols * 16, 2))
        ew_v = np.ndarray((C, P, s_cols), np.float16, buffer=blob2,
                          offset=A, strides=(NB2, s_cols * 2, 2))
        ew_v.fill(0)                        # pad slots must stay ew=0
        ew_v[cd, p_e, col] = ew16
        idx_lin = _buf("idx_lin", (C, s_cols * P), np.int16, zero=True)
        idx_lin[cd, epos] = rloc
        np.copyto(idxw_v, idx_lin.reshape(C, s_cols * 8, 16).transpose(0, 2, 1))
        return {"_blob2": blob2}, kt

    ew_slots = _buf("ew_slots", (C, P, s_cols), np.float16, zero=True)
    ew_slots[cd, p_e, col] = ew16
    idx_lin = _buf("idx_lin", (C, s_cols * P), np.int16, zero=True)
    idx_lin[cd, epos] = rloc
    idxw = _buf("idxw", (C, 16, s_cols * 8), np.int16)
    np.copyto(idxw, idx_lin.reshape(C, s_cols * 8, 16).transpose(0, 2, 1))

    arrays = dict(
        stage1,
        idxw=idxw.reshape(C * 16, s_cols * 8),
        ew=ew_slots.reshape(C * P, s_cols),
    )
    return arrays, kt


# ---------------------------------------------------------------- device build

def _dma_gather_raw(gp, out_ap, in_ap, idxs_ap, num_idxs, elem_size, elem_step,
                    queue_num):
    """dma_gather without the 256B elem_size restriction (non-transpose HBM
    path; the ucode only requires the row STRIDE to be a 256B multiple)."""
    assert idxs_ap.dtype == mybir.dt.int16
    assert in_ap.dtype == out_ap.dtype
    stride_bytes = elem_step * mybir.dt.size(in_ap.dtype)
    assert stride_bytes % 256 == 0
    stride_256 = stride_bytes // 256
    assert 0 < stride_256 < 256
    assert num_idxs % 4 == 0 and num_idxs <= 65535
    _in_ap = gp.lower_ap_dma(in_ap, for_custom_bir_dma=True)
    _idxs_ap = gp.lower_ap(idxs_ap)
    _out_ap = gp.lower_ap(out_ap)
    return gp.add_instruction(mybir.InstDMAGatherAnt(
        name=gp.bass.get_next_instruction_name(),
        ins=[*_in_ap, _idxs_ap, gp.lower_val_access(gp.to_reg(num_idxs))],
        outs=[_out_ap],
        transpose=False,
        num_idxs=num_idxs,
        elem_size=elem_size,
        stride_bytes_256=stride_256,
        gen_mode=0,
        single_packet=False,
        queue_num=queue_num,
        sbuf_tokens_per_rank=0,
        sbuf_free_dim_per_rank=0,
        sbuf_free_dim_pad_per_rank=0,
        sbuf_byte_offset=0,
    ))


def build_program(cfg, kt, n_queues=1):
    T, C = cfg.tiles, cfg.cores
    npc = cfg.npc
    npcp, nrows, SR = cfg.npcp, cfg.nrows, cfg.srows
    pieces = _pieces_from_kt(kt)
    s_cols = NSEC * sum(ws for _, _, ws in pieces)
    f16, f32, i16 = mybir.dt.float16, mybir.dt.float32, mybir.dt.int16

    nc = bacc.Bacc("TRN2", target_bir_lowering=False, debug=False,
                   enable_asserts=True, num_devices=C, num_swdge_queues=n_queues)

    xr = nc.dram_tensor("xr", [npc, D], f16, kind="ExternalInput")
    wt = nc.dram_tensor("wt", [D, D], f16, kind="ExternalInput")
    b64 = nc.dram_tensor("b64", [P, D], f32, kind="ExternalInput")
    dinvd = nc.dram_tensor("dinv", [P, T], f32, kind="ExternalInput")
    idxd = nc.dram_tensor("idxw", [16, s_cols * 8], i16, kind="ExternalInput")
    ewd = nc.dram_tensor("ew", [P, s_cols], f16, kind="ExternalInput")
    y = nc.dram_tensor("y", [npcp, D], f16, kind="ExternalOutput")

    ag_in = nc.dram_tensor("ag_in", [npcp, 2 * D], f16)
    table = nc.dram_tensor("table", [nrows, 2 * D], f16, addr_space="Shared")

    with tile.TileContext(nc) as tc:
        with (
            tc.tile_pool(name="const", bufs=1) as cp,
            tc.tile_pool(name="psum", bufs=4, space="PSUM") as pp,
            tc.tile_pool(name="xp", bufs=3) as xp,
            tc.tile_pool(name="mp", bufs=2) as mp,
            tc.tile_pool(name="ip", bufs=2) as ip,
        ):
            wt_sb = cp.tile([D, D], f16)
            id_sb = cp.tile([P, P], f16)
            b_sb = cp.tile([P, D], f32)
            dinv_sb = cp.tile([P, T], f32)
            ew_sb = cp.tile([P, s_cols], f16)
            h_sb = cp.tile([P, T * 2 * D], f16)
            oacc = cp.tile([P, T * D], f32)
            y_sb = cp.tile([P, T * D], f16)

            from concourse import library_config
            nc.gpsimd.load_library(library_config.mlp)

            nc.sync.dma_start(out=wt_sb[:], in_=wt.ap())
            nc.sync.dma_start(out=b_sb[:], in_=b64.ap())
            nc.sync.dma_start(out=dinv_sb[:], in_=dinvd.ap())
            nc.sync.dma_start(out=ew_sb[:], in_=ewd.ap())
            masks.make_identity(nc, id_sb[:])
            nc.vector.memset(h_sb[:], 0.0)

            # ---- h' = (x @ W^T) * dinv, fp16 rows at 256B pitch
            for t in range(T):
                rows = min(P, npc - t * P)
                xt_ld = xp.tile([P, D], f16, tag="xld")
                if rows < P:
                    nc.vector.memset(xt_ld[:], 0.0)
                nc.sync.dma_start(out=xt_ld[0:rows, :],
                                  in_=xr.ap()[t * P:t * P + rows, :])
                psT = pp.tile([D, P], f16, space="PSUM")
                nc.tensor.transpose(psT[:], xt_ld[:], id_sb[:])
                xtT = xp.tile([D, P], f16, tag="xtT")
                nc.any.tensor_copy(xtT[:], psT[:])
                psH = pp.tile([P, D], f32, space="PSUM")
                nc.tensor.matmul(psH[:], lhsT=xtT[:], rhs=wt_sb[:],
                                 start=True, stop=True)
                nc.scalar.activation(
                    out=h_sb[:, t * 2 * D:t * 2 * D + D], in_=psH[:],
                    func=mybir.ActivationFunctionType.Copy,
                    scale=dinv_sb[:, t:t + 1])

            # table rows in natural node order: row l = t*128+p
            nc.sync.dma_start(
                out=ag_in.ap().rearrange("(t p) f -> p t f", p=P),
                in_=h_sb[:].rearrange("p (t f) -> p t f", f=2 * D))
            nc.gpsimd.collective_compute(
                "AllGather", mybir.AluOpType.bypass,
                replica_groups=[list(range(C))],
                ins=[ag_in.ap().opt()], outs=[table.ap().opt()],
            )

            # ---- per piece: gather bands, * ew, segmented reduce
            for pi, (a, bnd, ws) in enumerate(pieces):
                colbase = sum(NSEC * pieces[q_][2] for q_ in range(pi))
                msgs = mp.tile([P, MAX_PIECE_COLS, D], f16, tag="msgs")
                idxt = ip.tile([P, MAX_PIECE_COLS * 8], i16, tag="idx")
                for kk in range(8):
                    nc.sync.dma_start(
                        out=idxt[16 * kk:16 * (kk + 1), 0:NSEC * ws * 8],
                        in_=idxd.ap()[:, colbase * 8:(colbase + NSEC * ws) * 8])
                for g in range(NSEC):
                    _dma_gather_raw(
                        nc.gpsimd,
                        out_ap=msgs[:, g * ws:(g + 1) * ws, :],
                        in_ap=table.ap()[g * SR:(g + 1) * SR, 0:D],
                        idxs_ap=idxt[:, g * ws * 8:(g + 1) * ws * 8],
                        num_idxs=P * ws,
                        elem_size=D,
                        elem_step=2 * D,
                        queue_num=g % n_queues,
                    )
                ewp = ew_sb[:, colbase:colbase + NSEC * ws]
                nc.vector.tensor_tensor(
                    out=msgs[:, 0:NSEC * ws, :], in0=msgs[:, 0:NSEC * ws, :],
                    in1=ewp[:, :, None].to_broadcast([P, NSEC * ws, D]),
                    op=mybir.AluOpType.mult)
                mview = msgs[:, 0:NSEC * ws, :].rearrange(
                    "p (g w) f -> p f g w", g=NSEC)
                off = 0
                for t in range(a, bnd):
                    nc.vector.tensor_reduce(
                        out=oacc[:, t * D:(t + 1) * D],
                        in_=mview[:, :, :, off:off + kt[t]],
                        axis=mybir.AxisListType.XY,
                        op=mybir.AluOpType.add,
                    )
                    off += kt[t]

            # ---- out = relu((acc + h'_own) * dinv + b)
            ov = oacc[:].rearrange("p (t f) -> p t f", f=D)
            hv = h_sb[:].rearrange("p (t f) -> p t f", f=2 * D)
            nc.vector.tensor_tensor(out=ov, in0=ov, in1=hv[:, :, 0:D],
                                    op=mybir.AluOpType.add)
            nc.vector.tensor_tensor(
                out=ov, in0=ov,
                in1=dinv_sb[:, :, None].to_broadcast([P, T, D]),
                op=mybir.AluOpType.mult)
            nc.vector.tensor_tensor(
                out=ov, in0=ov,
                in1=b_sb[:, None, :].to_broadcast([P, T, D]),
                op=mybir.AluOpType.add)
            nc.scalar.activation(y_sb[:], oacc[:],
                                 mybir.ActivationFunctionType.Relu)
            nc.sync.dma_start(
                out=y.ap().rearrange("(t p) f -> p t f", p=P),
                in_=y_sb[:].rearrange("p (t f) -> p t f", f=D))

    nc.compile()
    return nc


# ---------------------------------------------------------------- runner


BLOB0 = ("xr", "wt", "b64", "dinv")   # edge-independent, uploaded early
BLOB1 = ("idxw", "ew")                # edge-dependent
IN_ORDER = BLOB0 + BLOB1


def blob_layout(in_names, shapes):
    """Returns [(name, blob_id, byte_off, per-shard shape, dtype)]."""
    assert tuple(in_names) == IN_ORDER, in_names
    blob_of = {n: 0 for n in BLOB0}
    blob_of.update({n: 1 for n in BLOB1})
    offs = [0, 0]
    layout = []
    for name in in_names:
        shape, dtype = shapes[name]
        bid = blob_of[name]
        nbytes = int(np.prod(shape)) * np.dtype(dtype).itemsize
        layout.append((name, bid, offs[bid], shape, dtype))
        offs[bid] += nbytes
    return layout, offs


def _pack(arrays, names, n_cores, tag=None):
    """Concatenate per-core byte segments -> (n_cores, bytes) uint8."""
    segs = [np.ascontiguousarray(arrays[n]).view(np.uint8).reshape(
                n_cores, -1) for n in names]
    if tag is None:
        return np.concatenate(segs, axis=1)
    nb = sum(s.shape[1] for s in segs)
    out = _buf(tag, (n_cores, nb), np.uint8)
    off = 0
    for s in segs:
        out[:, off:off + s.shape[1]] = s
        off += s.shape[1]
    return out


_BUFS = {}


def _buf(name, shape, dtype, zero=False):
    """Reusable host buffer (avoids per-call mmap/page-fault churn on the
    multi-MB scratch arrays). Safe to reuse across kernel() calls: every
    device transfer is fully drained before kernel() returns."""
    key = (name, tuple(shape), np.dtype(dtype).str)
    a = _BUFS.get(key)
    if a is None:
        a = np.zeros(shape, dtype)
        _BUFS[key] = a
    elif zero:
        a.fill(0)
    return a


_GLOBAL = {}


def _shd():
    if "shd" not in _GLOBAL:
        import jax
        from jax.sharding import Mesh, PartitionSpec, NamedSharding
        devices = jax.devices()[:CORES]
        mesh = Mesh(np.asarray(devices), ("core",))
        _GLOBAL["mesh"] = mesh
        _GLOBAL["shd"] = NamedSharding(mesh, PartitionSpec("core"))
    return _GLOBAL["shd"]


class _Runner:
    """Persistent PJRT executor for one compiled program. Inputs arrive as
    two per-core uint8 blobs (one host->device transfer each); they are
    sliced/bitcast to the kernel's tensors on-device inside shard_map.
    The donated output buffer is the previous call's output."""

    def __init__(self, nc, n_cores, split=None):
        import jax
        import jax.numpy as jnp
        from jax import lax
        from jax.experimental.shard_map import shard_map
        from jax.sharding import Mesh, PartitionSpec, NamedSharding
        from concourse import bass2jax as B
        import concourse.mybir as mb

        B.install_neuronx_cc_hook()
        self.n_cores = n_cores
        self.split = split
        partition_name = (nc.partition_id_tensor.name
                          if nc.partition_id_tensor else None)
        in_names, out_names, out_avals = [], [], []
        shapes = {}
        for alloc in nc.m.functions[0].allocations:
            if not isinstance(alloc, mb.MemoryLocationSet):
                continue
            name = alloc.memorylocations[0].name
            if alloc.kind == "ExternalInput":
                if name != partition_name:
                    in_names.append(name)
                    shapes[name] = (tuple(alloc.tensor_shape),
                                    mb.dt.np(alloc.dtype))
            elif alloc.kind == "ExternalOutput":
                shape = tuple(alloc.tensor_shape)
                dtype = mb.dt.np(alloc.dtype)
                out_names.append(name)
                out_avals.append(jax.core.ShapedArray(shape, dtype))
        self.in_names = list(in_names)
        self.out_names = out_names
        self.out_avals = out_avals
        self.layout, self.blob_sizes = blob_layout(in_names, shapes)
        n_outs = len(out_avals)
        all_in_names = self.in_names + out_names
        if partition_name is not None:
            all_in_names.append(partition_name)

        def _body(*args):
            operands = list(args)
            if partition_name is not None:
                operands.append(B.partition_id_tensor())
            outs = B._bass_exec_p.bind(
                *operands,
                out_avals=tuple(out_avals),
                in_names=tuple(all_in_names),
                out_names=tuple(out_names),
                lowering_input_output_aliases=(),
                sim_require_finite=True,
                sim_require_nnan=True,
                nc=nc,
            )
            return tuple(outs)

        self.sharding = _shd()
        self.mesh = _GLOBAL["mesh"]
        n_params = len(self.in_names)
        in_specs = (PartitionSpec("core"),) * (n_params + n_outs)
        out_specs = (PartitionSpec("core"),) * n_outs
        self.fn = jax.jit(
            shard_map(_body, mesh=self.mesh, in_specs=in_specs,
                      out_specs=out_specs, check_rep=False),
            donate_argnums=tuple(range(n_params, n_params + n_outs)),
            keep_unused=True)

        nc_ = n_cores

        def _seg(blob, off, shape, dtype):
            it = np.dtype(dtype).itemsize
            nb = int(np.prod(shape)) * it
            seg = blob[:, off:off + nb]
            if it > 1:
                seg = lax.bitcast_convert_type(
                    seg.reshape(nc_, nb // it, it), dtype)
            return seg.reshape((nc_ * shape[0],) + tuple(shape[1:]))

        def _repack(b1, b2):
            blobs = (b1, b2)
            return tuple(_seg(blobs[bid], off, shape, dtype)
                         for name, bid, off, shape, dtype in self.layout)

        self.repack_fn = jax.jit(
            _repack, out_shardings=(self.sharding,) * n_params)

        if self.split is not None:
            wA, wB = self.split

            def _repack3(b0, ba, bb):
                args = []
                for name, bid, off, shape, dtype in self.layout:
                    if bid == 0:
                        args.append(_seg(b0, off, shape, dtype))
                ihalf, ehalf = [], []
                for blob, w in ((ba, wA), (bb, wB)):
                    ihalf.append(_seg(blob, 0, (16, w * 8), np.int16))
                    ehalf.append(_seg(blob, w * 256, (P, w), np.float16))
                args.append(jnp.concatenate(ihalf, axis=1))
                args.append(jnp.concatenate(ehalf, axis=1))
                return tuple(args)

            self.repack3_fn = jax.jit(
                _repack3, out_shardings=(self.sharding,) * n_params)

        zero_shapes = tuple((n_cores * a.shape[0], *a.shape[1:])
                            for a in out_avals)
        zero_dtypes = tuple(a.dtype for a in out_avals)

        def _mk_zeros():
            return tuple(jnp.zeros(s, d)
                         for s, d in zip(zero_shapes, zero_dtypes))

        self.zeros_fn = jax.jit(_mk_zeros, out_shardings=(self.sharding,) * n_outs)
        self._ybuf = None

    def execute_async(self, d1, d2):
        """Dispatch repack + kernel; returns the (device) output array
        without forcing a host copy."""
        if self._ybuf is None:
            self._ybuf = self.zeros_fn()[0]
        ins = self.repack_fn(d1, d2)
        ybuf, self._ybuf = self._ybuf, None
        outs = self.fn(*ins, ybuf)
        return outs[0]

    def execute3_async(self, d1, d2a, d2b):
        if self._ybuf is None:
            self._ybuf = self.zeros_fn()[0]
        ins = self.repack3_fn(d1, d2a, d2b)
        ybuf, self._ybuf = self._ybuf, None
        outs = self.fn(*ins, ybuf)
        return outs[0]

    def retire(self, y_dev):
        """Hand the fetched output back as the next call's donated buffer."""
        self._ybuf = y_dev


_CACHE = {}


def _split_of(kt):
    """(wA, wB) piece-boundary split widths, or None if only one piece."""
    pieces = _pieces_from_kt(kt)
    psplit = len(pieces) // 2
    if psplit < 1:
        return None
    s_cols = NSEC * sum(ws for _, _, ws in pieces)
    colsplit = NSEC * sum(pieces[q_][2] for q_ in range(psplit))
    return (colsplit, s_cols - colsplit)


def _get_runner(cfg, kt):
    key = (cfg.n, cfg.cores, kt)
    rkey = ("runner",) + key
    if rkey not in _CACHE:
        if key not in _CACHE:
            _CACHE[key] = build_program(cfg, kt)
        _CACHE[rkey] = _Runner(_CACHE[key], cfg.cores, split=_split_of(kt))
    return _CACHE[rkey]


def run(cfg, x, edge_index, edge_weight, W, b, use_sim=False):
    C, npc, npcp = cfg.cores, cfg.npc, cfg.npcp

    if use_sim:
        arrays, kt = host_prep(cfg, x, edge_index, edge_weight, W, b)
        key = (cfg.n, cfg.cores, kt)
        if key not in _CACHE:
            _CACHE[key] = build_program(cfg, kt)
        nc = _CACHE[key]
        from concourse import bass_interp
        sim = bass_interp.MultiCoreSim(nc, num_cores=C)
        for c in range(C):
            for k, v in arrays.items():
                rows = v.shape[0] // C
                sim.cores[c].tensor(k)[:] = v[c * rows:(c + 1) * rows]
            sim.cores[c].tensor("partition_id")[:] = np.int32(c)
        sim.simulate(check_with_hw=False)
        y16 = np.stack([np.asarray(sim.cores[c].mem_tensor("y"))
                        for c in range(C)])
        full = np.empty((cfg.n, D), np.float32)
        for c in range(C):
            full[c * npc:(c + 1) * npc] = y16[c, :npc]
        return full

    import jax
    holder = {}

    d2s = {}

    def cb(s1):
        holder["d1"] = jax.device_put(_pack(s1, BLOB0, C, tag="blob0"), _shd())

    def hcb(h, blob):
        d2s[h] = jax.device_put(blob, _shd())

    # half_cb=hcb (split upload) measured slower on this tunnel: the extra
    # device_put round-trips and masked gathers outweigh the overlap.
    res, kt = host_prep(cfg, x, edge_index, edge_weight, W, b,
                        stage1_cb=cb, half_cb=None)
    runner = _get_runner(cfg, kt)
    if "_blob2" in res:
        d2 = jax.device_put(res["_blob2"], _shd())
        y_dev = runner.execute_async(holder["d1"], d2)
    elif isinstance(res, dict):
        d2 = jax.device_put(_pack(res, BLOB1, C, tag="blob1"), _shd())
        y_dev = runner.execute_async(holder["d1"], d2)
    else:
        y_dev = runner.execute3_async(holder["d1"], d2s[0], d2s[1])

    # fetch per-shard in threads, casting each into place as it lands
    from concurrent.futures import ThreadPoolExecutor
    full = np.empty((cfg.n, D), np.float32)
    shards = sorted(y_dev.addressable_shards,
                    key=lambda s: s.index[0].start or 0)

    def _land(cs):
        c, s = cs
        h = np.asarray(s.data)
        full[c * npc:(c + 1) * npc] = h[:npc]

    with ThreadPoolExecutor(4) as ex:
        list(ex.map(_land, enumerate(shards)))
    runner.retire(y_dev)
    return full


def kernel(x, edge_index, edge_weight, W, b):
    cfg = Cfg(100000)
    return run(cfg, x, edge_index, edge_weight, W, b)
